# revision 1
# baseline (speedup 1.0000x reference)
"""Trainium2 Bass kernel for DGN-style GNN message passing (3x NNConv + pairwise L1 CBT).

Strategy (8 NeuronCores, SPMD single program, per-core differences in data):
 - Edges are sharded by destination node: core c owns nodes [64c, 64c+64) and all
   edges targeting them (host groups/pads edge lists to a common chunk count).
 - Per 128-edge chunk: PE computes the edge-MLP pre-activation z = eaT5.T @ lw5
   (bias folded in via a ones row), one DVE scalar_tensor_tensor applies
   relu(z) * h[src] straight out of PSUM (valid since h >= 0), and PE scatter-
   accumulates agg'[64, cin*cout] with a host-built 0/1 mask matmul. The sum
   over cin is deferred to one strided tensor_reduce per layer.
 - h[src] gathers use gpsimd indirect DMA with host-built row-index tables.
 - Between layers, h is shared with an AllGather; the CBT block computes each
   core's 64 output rows locally (fused |a-b| reduce), host concatenates.
"""
import os
import sys

for _p in ("/opt/trn_rl_repo", os.path.expanduser("~/.axon_site/_ro/trn_rl_repo")):
    if os.path.isdir(_p) and _p not in sys.path:
        sys.path.insert(0, _p)

import numpy as np

import concourse.bass as bass
import concourse.bacc as bacc
import concourse.tile as tile
from concourse import mybir
from concourse.bass import IndirectOffsetOnAxis
from concourse.bass_utils import run_bass_kernel_spmd

F32 = mybir.dt.float32
I32 = mybir.dt.int32
ALU = mybir.AluOpType
AXL = mybir.AxisListType

V = 4
DIMS = [(1, 36), (36, 24), (24, 8)]
P = 128
SLAB = 16


def _ap(t, dims, pdim=None):
    """AP with explicit (step, count) free dims; partition dim from `t` or override."""
    p0 = list(t.ap[0]) if pdim is None else [pdim[0], pdim[1]]
    return bass.AP(tensor=t.tensor, offset=t.offset, ap=[p0] + [[s, c] for s, c in dims])


def _host_prep(x, edge_attr, edge_index, n_cores):
    src = np.asarray(edge_index[0]).astype(np.int64)
    dst = np.asarray(edge_index[1]).astype(np.int64)
    ea = np.asarray(edge_attr, dtype=np.float32)
    nn = int(np.asarray(x).shape[0])
    npc = nn // n_cores

    cnt = np.bincount(dst, minlength=nn).astype(np.float32)
    recip = (1.0 / np.maximum(cnt, 1.0)).astype(np.float32)

    perm = np.argsort(dst, kind="stable")
    src_s, dst_s = src[perm], dst[perm]
    ea_s = ea[perm]
    bounds = np.searchsorted(dst_s, np.arange(0, nn + 1, npc))
    n_chunks = max(2, int(np.ceil(np.diff(bounds).max() / P)))
    e_pad = n_chunks * P

    cores = []
    for c in range(n_cores):
        lo, hi = int(bounds[c]), int(bounds[c + 1])
        k = hi - lo
        ea_t5 = np.zeros((5, e_pad), dtype=np.float32)
        ea_t5[:4, :k] = ea_s[lo:hi].T
        ea_t5[4, :k] = 1.0
        srcc = np.zeros((e_pad,), dtype=np.int32)
        srcc[:k] = src_s[lo:hi]
        dloc = np.zeros((e_pad,), dtype=np.int64)
        dloc[:k] = dst_s[lo:hi] - c * npc
        # PSUM matmul bases must be 0/32/64-aligned; narrow windows need host
        # repacking (see work/ notes) — use full-width masks (verified on HW).
        wstart = np.zeros(n_chunks, dtype=np.int64)
        wwidth = npc
        ar = np.arange(e_pad)
        rel = np.where(ar < k, dloc - wstart[ar // P], 0)
        if not ((rel[ar < k] >= 0).all() and (rel[ar < k] < wwidth).all()):
            wstart = np.zeros(n_chunks, dtype=np.int64)
            wwidth = npc
            rel = dloc
        masks = np.zeros((n_chunks, P, wwidth), dtype=np.float32)
        masks[ar // P, ar % P, rel] = (ar < k).astype(np.float32)
        ccnt = np.maximum(cnt[c * npc:(c + 1) * npc], 1.0).astype(np.float32)
        xloc = np.asarray(x, np.float32)[c * npc:(c + 1) * npc].reshape(npc)
        cores.append(
            dict(
                ea_t5=ea_t5,
                offs_t=np.ascontiguousarray(srcc.reshape(n_chunks, P).T),
                masks_t=np.ascontiguousarray(masks.transpose(1, 0, 2)),
                wstart=wstart,
                wwidth=wwidth,
                recip=recip[c * npc:(c + 1) * npc].reshape(-1, 1).copy(),
                cntrow=ccnt.reshape(1, npc).copy(),
                cntrep=np.ascontiguousarray(np.broadcast_to(ccnt[None, :], (64, npc))).copy(),
                xcnt_t=(xloc * ccnt).reshape(1, npc).copy(),
            )
        )
    ws0 = cores[0]["wstart"]
    assert all(np.array_equal(d["wstart"], ws0) and d["wwidth"] == cores[0]["wwidth"] for d in cores)
    return cores, n_chunks


def _build_program(nn, n_cores, n_chunks, x0, wstart=None, wwidth=None):
    npc = nn // n_cores
    nc = bacc.Bacc()
    e_pad = n_chunks * P

    ea_d = nc.declare_dram_parameter("ea_t5", [5, e_pad], F32, isOutput=False)
    offs_d = nc.declare_dram_parameter("offs_t", [P, n_chunks], I32, isOutput=False)
    if wwidth is None:
        wwidth = npc
        wstart = np.zeros(n_chunks, dtype=np.int64)
    masks_d = nc.declare_dram_parameter("masks_t", [P, n_chunks, wwidth], F32, isOutput=False)
    recip_d = nc.declare_dram_parameter("recip", [npc, 1], F32, isOutput=False)
    cntrow_d = nc.declare_dram_parameter("cntrow", [1, npc], F32, isOutput=False)
    cntrep_d = nc.declare_dram_parameter("cntrep", [64, npc], F32, isOutput=False)
    xcnt_d = nc.declare_dram_parameter("xcnt_t", [1, npc], F32, isOutput=False)
    lw_d, root_d, bias_d = [], [], []
    for li, (cin, cout) in enumerate(DIMS):
        lw_d.append(nc.declare_dram_parameter(f"lw5_{li}", [5, cin * cout], F32, isOutput=False))
        root_d.append(nc.declare_dram_parameter(f"root_{li}", [cin, cout], F32, isOutput=False))
        bias_d.append(nc.declare_dram_parameter(f"bias_{li}", [1, cout], F32, isOutput=False))
    out_d = nc.declare_dram_parameter("out_cbt", [npc, nn], F32, isOutput=True)
    dbgh_d = nc.declare_dram_parameter("dbg_h", [nn, DIMS[-1][1]], F32, isOutput=True)

    with tile.TileContext(nc) as tc:
        with (
            tc.tile_pool(name="consts", bufs=1) as consts,
            tc.tile_pool(name="ea", bufs=2) as ea_pool,
            tc.tile_pool(name="mk", bufs=2) as mk_pool,
            tc.tile_pool(name="off", bufs=2) as off_pool,
            tc.tile_pool(name="hs", bufs=3) as hs_pool,
            tc.tile_pool(name="pp", bufs=3) as p_pool,
            tc.tile_pool(name="sm", bufs=2) as sm_pool,
            tc.tile_pool(name="zp", bufs=2, space="PSUM") as z_pool,
            tc.tile_pool(name="ag", bufs=1, space="PSUM") as ag_pool,
            tc.tile_pool(name="dr", bufs=1, space="DRAM") as dram,
        ):
            # ---- constants ----
            lw_s, root_s, bias_s = [], [], []
            for li, (cin, cout) in enumerate(DIMS):
                t = consts.tile([5, cin * cout], F32, name=f"lw5s_{li}")
                nc.sync.dma_start(out=t, in_=lw_d[li][:, :])
                lw_s.append(t)
                r = consts.tile([cin, cout], F32, name=f"roots_{li}")
                nc.sync.dma_start(out=r, in_=root_d[li][:, :])
                root_s.append(r)
                b = consts.tile([1, cout], F32, name=f"biass_{li}")
                nc.sync.dma_start(out=b, in_=bias_d[li][:, :])
                bias_s.append(b)
            recip_s = consts.tile([npc, 1], F32)
            nc.sync.dma_start(out=recip_s, in_=recip_d[:, :])
            cntrow_s = consts.tile([1, npc], F32)
            nc.sync.dma_start(out=cntrow_s, in_=cntrow_d[:, :])
            cntrep_s = consts.tile([64, npc], F32)
            nc.sync.dma_start(out=cntrep_s, in_=cntrep_d[:, :])
            xcnt_s = consts.tile([1, npc], F32)
            nc.sync.dma_start(out=xcnt_s, in_=xcnt_d[:, :])
            zrow_s = consts.tile([1, 1024], F32)
            nc.vector.memset(zrow_s, 0.0)
            zcol_s = consts.tile([1, npc], F32)
            nc.vector.memset(zcol_s, 0.0)

            h_loc = [dram.tile([npc, cout], F32, name=f"hloc_{li}") for li, (_, cout) in enumerate(DIMS)]
            h_full = [dram.tile([nn, cout], F32, name=f"hfull_{li}") for li, (_, cout) in enumerate(DIMS)]

            h_prev_s = None
            for li, (cin, cout) in enumerate(DIMS):
                cc = cin * cout
                splits = [(o, min(512, cc - o)) for o in range(0, cc, 512)]
                aggp = ag_pool.tile([npc, cin, cout], F32, tag=f"agg{li}", name=f"aggp_{li}")
                aggf = aggp.rearrange("p i o -> p (i o)")
                for off, n in splits:
                    nc.tensor.matmul(
                        aggf[:, off:off + n], lhsT=zcol_s, rhs=zrow_s[:, :n],
                        start=True, stop=False,
                    )
                if li == 0:
                    lhst_prev = xcnt_s
                else:
                    h_t = sm_pool.tile([64, npc], F32, tag="ht", name=f"ht_{li}")
                    nc.sync.dma_start(
                        out=h_t[:cin, :],
                        in_=_ap(h_loc[li - 1][:, :], [(cin, npc)], pdim=(1, cin)),
                    )
                    h_tc = sm_pool.tile([64, npc], F32, tag="htc", name=f"htc_{li}")
                    nc.vector.tensor_tensor(
                        out=h_tc[:cin, :], in0=h_t[:cin, :], in1=cntrep_s[:cin, :], op=ALU.mult
                    )
                    lhst_prev = h_tc[:cin, :]
                ch = 0
                pending = None
                for s0 in range(0, n_chunks, SLAB):
                    sl = min(SLAB, n_chunks - s0)
                    ea_sl = ea_pool.tile([5, SLAB * P], F32, tag="ea", name=f"easl_{li}_{s0}")
                    nc.sync.dma_start(out=ea_sl[:, : sl * P], in_=ea_d[:, s0 * P:(s0 + sl) * P])
                    mk_sl = mk_pool.tile([P, SLAB, wwidth], F32, tag="mk", name=f"mksl_{li}_{s0}")
                    nc.sync.dma_start(out=mk_sl[:, :sl, :], in_=masks_d[:, s0:s0 + sl, :])
                    if li > 0:
                        of_sl = off_pool.tile([P, SLAB], I32, tag="of", name=f"ofsl_{li}_{s0}")
                        nc.sync.dma_start(out=of_sl[:, :sl], in_=offs_d[:, s0:s0 + sl])
                        hs_sl = hs_pool.tile([P, SLAB, cin], F32, tag="hs", name=f"hssl_{li}_{s0}")
                        if os.environ.get("K_ABLATE_GATHER"):
                            nc.sync.dma_start(
                                out=hs_sl.rearrange("p s c -> p (s c)"),
                                in_=_ap(h_full[li - 1][:, :], [(1, SLAB * cin)], pdim=(0, P)),
                            )
                        else:
                            for si in range(sl):
                                nc.gpsimd.indirect_dma_start(
                                    out=hs_sl[:, si, :],
                                    out_offset=None,
                                    in_=h_full[li - 1][:, :],
                                    in_offset=IndirectOffsetOnAxis(ap=of_sl[:, si:si + 1], axis=0),
                                )
                    for s in range(sl):
                        z = z_pool.tile([P, cin, cout], F32, tag="z", name=f"z_{li}_{s0}_{s}")
                        zf = z.rearrange("p i o -> p (i o)")
                        for off, n in ([(0, cout)] if os.environ.get("K_ABLATE_ZN") else splits):
                            nc.tensor.matmul(
                                zf[:, off:off + n],
                                lhsT=ea_sl[:, (s * P):(s + 1) * P],
                                rhs=lw_s[li][:, off:off + n],
                                start=True,
                                stop=True,
                            )
                        p_t = p_pool.tile([P, cin, cout], F32, tag="p", name=f"p_{li}_{s0}_{s}")
                        if os.environ.get("K_ABLATE_STT"):
                            nc.vector.tensor_scalar(
                                p_t.rearrange("p i o -> p (i o)"),
                                z.rearrange("p i o -> p (i o)"), 0.0, None, ALU.max)
                        elif li == 0:
                            nc.vector.tensor_scalar(p_t[:], z[:], 0.0, None, ALU.max)
                        else:
                            hsv = hs_sl[:, s, :]
                            nc.vector.scalar_tensor_tensor(
                                out=p_t[:],
                                in0=z[:],
                                scalar=0.0,
                                in1=_ap(hsv, [(1, cin), (0, cout)]),
                                op0=ALU.max,
                                op1=ALU.mult,
                            )
                        pf = p_t.rearrange("p i o -> p (i o)")
                        if pending is not None:
                            ppf, pmk, pch = pending
                            pw = int(wstart[pch])
                            for off, n in ([(0, cout)] if os.environ.get("K_ABLATE_SC") else splits):
                                nc.tensor.matmul(
                                    aggf[pw:pw + wwidth, off:off + n], lhsT=pmk,
                                    rhs=ppf[:, off:off + n],
                                    start=False, stop=False,
                                )
                        pending = (pf, mk_sl[:, s, :], ch)
                        ch += 1
                # ---- layer end: injections, then the final (group-closing) scatter ----
                nc.tensor.matmul(aggf[:, 0:cout], lhsT=lhst_prev, rhs=root_s[li], start=False, stop=False)
                nc.tensor.matmul(aggf[:, 0:cout], lhsT=cntrow_s, rhs=bias_s[li], start=False, stop=False)
                ppf, pmk, pch = pending
                pw = int(wstart[pch])
                for off, n in ([(0, cout)] if os.environ.get("K_ABLATE_SC") else splits):
                    nc.tensor.matmul(
                        aggf[pw:pw + wwidth, off:off + n], lhsT=pmk, rhs=ppf[:, off:off + n],
                        start=False, stop=False,
                    )
                for off, n in splits:
                    nc.tensor.matmul(
                        aggf[:, off:off + n], lhsT=zcol_s, rhs=zrow_s[:, :n],
                        start=False, stop=True,
                    )
                red = sm_pool.tile([npc, cout], F32, tag="red", name=f"red_{li}")
                nc.vector.tensor_reduce(
                    out=red,
                    in_=_ap(aggf, [(1, cout), (cout, 1 if os.environ.get("K_ABLATE_SC") else cin)]),
                    axis=AXL.X,
                    op=ALU.add,
                )
                h_s = sm_pool.tile([npc, cout], F32, tag=f"h{li}", name=f"h_{li}")
                nc.vector.tensor_scalar(h_s[:], red[:], recip_s[:, 0:1], 0.0, ALU.mult, ALU.max)
                nc.sync.dma_start(out=h_loc[li][:, :], in_=h_s[:])
                nc.gpsimd.collective_compute(
                    "AllGather",
                    ALU.bypass,
                    replica_groups=[list(range(n_cores))],
                    ins=[h_loc[li].opt()],
                    outs=[h_full[li].opt()],
                )
                h_prev_s = h_s

            # ---- CBT ----
            dlast = DIMS[-1][1]
            hrep = sm_pool.tile([npc, nn, dlast], F32, tag="hrep")
            nc.sync.dma_start(
                out=hrep.rearrange("p j d -> p (j d)"),
                in_=_ap(h_full[-1][:, :], [(1, nn * dlast)], pdim=(0, npc)),
            )
            diff = sm_pool.tile([npc, nn, dlast], F32, tag="diff")
            nc.vector.tensor_tensor(
                out=diff[:],
                in0=hrep[:],
                in1=_ap(h_prev_s[:], [(0, nn), (1, dlast)]),
                op=ALU.subtract,
            )
            cbt = sm_pool.tile([npc, nn], F32, tag="cbt")
            nc.vector.tensor_reduce(
                out=cbt, in_=diff[:], axis=AXL.X, op=ALU.add, apply_absolute_value=True
            )
            nc.sync.dma_start(out=out_d[:, :], in_=cbt[:])
            nc.sync.dma_start(out=dbgh_d[:, :], in_=h_full[-1][:, :])
    return nc


def _run(inputs, n_cores, sim=False):
    x = np.asarray(inputs["x"], np.float32)
    nn = x.shape[0]
    x0 = float(x[0, 0])
    assert np.all(x == x0) and x0 >= 0.0, "general-x path not implemented"
    cores, n_chunks = _host_prep(x, inputs["edge_attr"], inputs["edge_index"], n_cores)
    nc = _build_program(nn, n_cores, n_chunks, x0, cores[0]["wstart"], cores[0]["wwidth"])
    nc.finalize()

    in_maps = []
    for c in range(n_cores):
        m = dict(
            ea_t5=cores[c]["ea_t5"],
            offs_t=cores[c]["offs_t"],
            masks_t=cores[c]["masks_t"],
            recip=cores[c]["recip"],
            cntrow=cores[c]["cntrow"],
            cntrep=cores[c]["cntrep"],
            xcnt_t=cores[c]["xcnt_t"],
        )
        for li, (cin, cout) in enumerate(DIMS):
            lw = np.asarray(inputs[f"lin_w{li + 1}"], np.float32)
            lb = np.asarray(inputs[f"lin_b{li + 1}"], np.float32)
            lw5 = np.vstack([lw, lb[None, :]]).astype(np.float32)
            if li == 0:
                lw5 = lw5 * x0
            m[f"lw5_{li}"] = lw5
            m[f"root_{li}"] = np.asarray(inputs[f"root{li + 1}"], np.float32)
            m[f"bias_{li}"] = np.asarray(inputs[f"bias{li + 1}"], np.float32).reshape(1, -1)
        in_maps.append(m)

    global _LAST
    _LAST = (nc, in_maps)
    if sim:
        from concourse.bass_interp import MultiCoreSim

        ms = MultiCoreSim(nc, n_cores)
        for c in range(n_cores):
            for k, v in in_maps[c].items():
                ms.cores[c].tensor(k)[:] = v
        ms.simulate()
        rows = [np.asarray(ms.cores[c].tensor("out_cbt")) for c in range(n_cores)]
    else:
        res = run_bass_kernel_spmd(nc, in_maps, list(range(n_cores)))
        rows = [res.results[c]["out_cbt"] for c in range(n_cores)]
    return np.concatenate(rows, 0).astype(np.float32)


_LAST = None


def kernel(**inputs) -> np.ndarray:
    return _run(inputs, n_cores=8, sim=False)



# revision 14
# speedup vs baseline: 3.2076x; 3.2076x over previous
"""Trainium2 Bass kernel for DGN-style GNN message passing (3x NNConv + pairwise L1 CBT).

Strategy (8 NeuronCores, SPMD, edges sharded by destination node):
 - Core c owns nodes [64c, 64c+64) and all edges targeting them (host sorts
   edges by dst, pads per-core lists to a common chunk count; 128 edges/chunk).
 - All hot tensors are bf16 (PE matmuls run 4x faster than fp32 in the cost
   model; DVE gets 2x on packed bf16 SBUF ops). PSUM accumulation stays fp32.
 - Edge-MLP weights use an [o,i] (cout-major) column order so the per-edge
   h[src] multiply broadcasts over o with a packed-i last dim (DVE 2x mode).
 - ea is packed 3 chunks per 128-col group at partition bases {0,32,64}
   (matmul base-partition constraint) so the one-time DMA is ~8us, and layers
   1/3 compute z for 3 chunks in ONE matmul via a block-diagonal lw.
 - Per 128-edge chunk (layer 2): PE z matmul -> PSUM fp32; drain split between
   DVE (fused relu*h scalar_tensor_tensor on ~1/4 cols) and ACT (relu) + DVE
   (2x bf16 multiply); PE mask-matmul scatter-accumulates into agg PSUM.
 - Root/bias injections go into spare agg columns [cc:cc+cout]; one strided
   tensor_reduce folds the cin sum; scatter-mean via reciprocal-count multiply.
 - h is AllGathered in bf16 between layers; the 15us collective latency is
   hidden by emitting the next layer's (h-independent) z matmuls + ACT relus
   ahead with deep prelu buffering. h[src] gathers are slab-batched indirect
   DMAs on gpsimd.
 - CBT block: each core computes its 64 output rows from the gathered h3.
"""
import os
import sys

for _p in ("/opt/trn_rl_repo", os.path.expanduser("~/.axon_site/_ro/trn_rl_repo")):
    if os.path.isdir(_p) and _p not in sys.path:
        sys.path.insert(0, _p)

import numpy as np

import concourse.bass as bass
import concourse.bacc as bacc
import concourse.tile as tile
from concourse import mybir
from concourse.bass import IndirectOffsetOnAxis
from concourse.bass_utils import run_bass_kernel_spmd

F32 = mybir.dt.float32
BF16 = mybir.dt.bfloat16
I32 = mybir.dt.int32
ALU = mybir.AluOpType
AXL = mybir.AxisListType
RELU = mybir.ActivationFunctionType.Relu
NPBF = mybir.dt.np(BF16)

V = 4
DIMS = [(1, 36), (36, 24), (24, 8)]
P = 128
SLAB = 16
NPRO2 = 24     # layer-2 prologue depth (chunks) hiding the AllGather
NCATCH2 = 36   # layer-2 chunks after prologue that stay in ACT-full mode
STT2 = 6       # layer-2 o-groups handled by fused DVE stt (of cout=24)
NPRO3 = 24     # layer-3 prologue depth (triples)
NCATCH3 = 32
STT3 = 2       # layer-3 o-groups (of cout=8) per chunk on DVE stt


def _ap(t, dims, pdim=None):
    p0 = list(t.ap[0]) if pdim is None else [pdim[0], pdim[1]]
    return bass.AP(tensor=t.tensor, offset=t.offset, ap=[p0] + [[s, c] for s, c in dims])


def _apo(t, off, dims):
    """AP into tile/AP `t` at free-element offset `off` with explicit free dims."""
    return bass.AP(tensor=t.tensor, offset=t.offset + off,
                   ap=[list(t.ap[0])] + [[s, c] for s, c in dims])


def _host_prep(x, edge_attr, edge_index, n_cores):
    src = np.asarray(edge_index[0]).astype(np.int64)
    dst = np.asarray(edge_index[1]).astype(np.int64)
    ea = np.asarray(edge_attr, dtype=np.float32)
    nn = int(np.asarray(x).shape[0])
    npc = nn // n_cores

    cnt = np.bincount(dst, minlength=nn).astype(np.float32)
    recip = (1.0 / np.maximum(cnt, 1.0)).astype(np.float32)

    perm = np.argsort(dst, kind="stable")
    src_s, dst_s = src[perm], dst[perm]
    ea_s = ea[perm]
    bounds = np.searchsorted(dst_s, np.arange(0, nn + 1, npc))
    n_chunks = int(np.ceil(np.diff(bounds).max() / P))
    n_chunks = max(3, 3 * int(np.ceil(n_chunks / 3)))  # pad to triple multiple
    e_pad = n_chunks * P
    cpb = n_chunks // 3

    cores = []
    for c in range(n_cores):
        lo, hi = int(bounds[c]), int(bounds[c + 1])
        k = hi - lo
        # ea69: chunk c -> partition base 32*(c%3), col group c//3
        ea5 = np.zeros((5, e_pad), dtype=np.float32)
        ea5[:4, :k] = ea_s[lo:hi].T
        ea5[4, :k] = 1.0
        ea69 = np.zeros((69, cpb * P), dtype=NPBF)
        for b in range(3):
            # chunks with c%3==b laid out at rows 32b..32b+5
            blk = ea5.reshape(5, n_chunks, P)[:, b::3, :].reshape(5, cpb * P)
            ea69[32 * b:32 * b + 5, :] = blk.astype(NPBF)
        srcc = np.zeros((e_pad,), dtype=np.int32)
        srcc[:k] = src_s[lo:hi]
        dloc = np.zeros((e_pad,), dtype=np.int64)
        dloc[:k] = dst_s[lo:hi] - c * npc
        ar = np.arange(e_pad)
        masks = np.zeros((n_chunks, P, npc), dtype=NPBF)
        masks[ar // P, ar % P, dloc] = (ar < k).astype(NPBF)
        ccnt = np.maximum(cnt[c * npc:(c + 1) * npc], 1.0).astype(np.float32)
        xloc = np.asarray(x, np.float32)[c * npc:(c + 1) * npc].reshape(npc)
        cores.append(
            dict(
                ea69=ea69,
                offs_t=np.ascontiguousarray(srcc.reshape(n_chunks, P).T),
                masks_t=np.ascontiguousarray(masks.transpose(1, 0, 2)),
                recip=recip[c * npc:(c + 1) * npc].reshape(-1, 1).copy(),
                cntrow=ccnt.reshape(1, npc).astype(NPBF),
                cntrep=np.ascontiguousarray(
                    np.broadcast_to(ccnt[None, :], (36, npc))).astype(NPBF),
                xcnt=(xloc * ccnt).reshape(1, npc).astype(NPBF),
            )
        )
    return cores, n_chunks


def _perm_oi(lw5, cin, cout):
    """[5, cin*cout] in (i,o) order -> (o,i) order."""
    return np.ascontiguousarray(
        lw5.reshape(5, cin, cout).transpose(0, 2, 1).reshape(5, cin * cout))


def _build_program(nn, n_cores, n_chunks):
    npc = nn // n_cores
    nc = bacc.Bacc()
    cpb = n_chunks // 3
    n_slabs = (n_chunks + SLAB - 1) // SLAB

    ea_d = nc.declare_dram_parameter("ea69", [69, cpb * P], BF16, isOutput=False)
    offs_d = nc.declare_dram_parameter("offs_t", [P, n_chunks], I32, isOutput=False)
    masks_d = nc.declare_dram_parameter("masks_t", [P, n_chunks, npc], BF16, isOutput=False)
    recip_d = nc.declare_dram_parameter("recip", [npc, 1], F32, isOutput=False)
    cntrow_d = nc.declare_dram_parameter("cntrow", [1, npc], BF16, isOutput=False)
    cntrep_d = nc.declare_dram_parameter("cntrep", [36, npc], BF16, isOutput=False)
    xcnt_d = nc.declare_dram_parameter("xcnt", [1, npc], BF16, isOutput=False)
    lwr_d, lwbd_d, root_d, bias_d = [], [], [], []
    for li, (cin, cout) in enumerate(DIMS):
        cc = cin * cout
        lwr_d.append(nc.declare_dram_parameter(f"lwr_{li}", [69, cc], BF16, isOutput=False))
        lwbd_d.append(nc.declare_dram_parameter(f"lwbd_{li}", [69, 3 * cc], BF16, isOutput=False))
        root_d.append(nc.declare_dram_parameter(f"root_{li}", [cin, cout], BF16, isOutput=False))
        bias_d.append(nc.declare_dram_parameter(f"bias_{li}", [1, cout], BF16, isOutput=False))
    out_d = nc.declare_dram_parameter("out_cbt", [npc, nn], F32, isOutput=True)

    CC2 = DIMS[1][0] * DIMS[1][1]       # 864
    SPLITS2 = [(0, 512), (512, CC2 - 512)]

    with tile.TileContext(nc) as tc:
        with (
            tc.tile_pool(name="consts", bufs=1) as consts,
            tc.tile_pool(name="hs", bufs=3) as hs_pool,
            tc.tile_pool(name="pre2", bufs=NPRO2 + 4) as pre2_pool,
            tc.tile_pool(name="pre3", bufs=NPRO3 + 4) as pre3_pool,
            tc.tile_pool(name="pp", bufs=8) as p_pool,
            tc.tile_pool(name="sm", bufs=1) as sm_pool,
            tc.tile_pool(name="zp", bufs=3, space="PSUM") as z_pool,
            tc.tile_pool(name="ag", bufs=1, space="PSUM") as ag_pool,
            tc.tile_pool(name="dr", bufs=1, space="DRAM") as dram,
        ):
            # ---- one-time constants ----
            lwr_s, lwbd_s, root_s, bias_s = [], [], [], []
            for li, (cin, cout) in enumerate(DIMS):
                cc = cin * cout
                if li == 1:
                    t = consts.tile([69, cc], BF16, name=f"lwr_{li}")
                    nc.sync.dma_start(out=t, in_=lwr_d[li][:, :])
                else:
                    t = None
                lwr_s.append(t)
                if li != 1:
                    t = consts.tile([69, 3 * cc], BF16, name=f"lwbd_{li}")
                    nc.sync.dma_start(out=t, in_=lwbd_d[li][:, :])
                else:
                    t = None
                lwbd_s.append(t)
                r = consts.tile([cin, cout], BF16, name=f"root_{li}")
                nc.sync.dma_start(out=r, in_=root_d[li][:, :])
                root_s.append(r)
                b = consts.tile([1, cout], BF16, name=f"bias_{li}")
                nc.sync.dma_start(out=b, in_=bias_d[li][:, :])
                bias_s.append(b)
            recip_s = consts.tile([npc, 1], F32)
            nc.sync.dma_start(out=recip_s, in_=recip_d[:, :])
            cntrow_s = consts.tile([1, npc], BF16)
            nc.sync.dma_start(out=cntrow_s, in_=cntrow_d[:, :])
            cntrep_s = consts.tile([36, npc], BF16)
            nc.sync.dma_start(out=cntrep_s, in_=cntrep_d[:, :])
            xcnt_s = consts.tile([1, npc], BF16)
            nc.sync.dma_start(out=xcnt_s, in_=xcnt_d[:, :])
            offs_s = consts.tile([P, n_chunks], I32)
            nc.sync.dma_start(out=offs_s, in_=offs_d[:, :])

            # ea69: split into 8 column pieces, alternate SP / gpsimd queues
            ea_s = consts.tile([69, cpb * P], BF16, name="ea69")
            npieces = 8
            cols = cpb * P
            step = ((cols // npieces) // P) * P
            starts = list(range(0, cols, step))
            for pi, s0 in enumerate(starts):
                s1 = min(cols, s0 + step)
                eng = nc.sync if pi == 0 else nc.gpsimd
                eng.dma_start(out=ea_s[:, s0:s1], in_=ea_d[:, s0:s1])

            # masks: one const tile per slab
            mask_s = []
            for sl in range(n_slabs):
                c0 = sl * SLAB
                c1 = min(n_chunks, c0 + SLAB)
                t = consts.tile([P, c1 - c0, npc], BF16, name=f"mask_{sl}")
                nc.sync.dma_start(out=t, in_=masks_d[:, c0:c1, :])
                mask_s.append(t)

            h_loc = [dram.tile([npc, cout], BF16, name=f"hloc_{li}")
                     for li, (_, cout) in enumerate(DIMS)]
            h_full = [dram.tile([nn, cout], BF16, name=f"hfull_{li}")
                      for li, (_, cout) in enumerate(DIMS)]

            agg = ag_pool.tile([npc, 928], F32, tag="agg", name="agg")
            h_prev_bf = None

            def mask_ap(c):
                sl, si = c // SLAB, c % SLAB
                return mask_s[sl][:, si, :]

            def z_lhsT(c):
                b, t = c % 3, c // 3
                return ea_s[32 * b:32 * b + 5, t * P:(t + 1) * P]

            def epilogue(li, cin, cout, cc):
                # injections: (h*cnt)@root + cnt (x) bias into agg[:, cc:cc+cout]
                if li == 0:
                    lhst = xcnt_s
                else:
                    h_t = sm_pool.tile([36, npc], BF16, tag="ht", name=f"ht_{li}")
                    nc.sync.dma_start(
                        out=h_t[:cin, :],
                        in_=_ap(h_loc[li - 1][:, :], [(cin, npc)], pdim=(1, cin)),
                    )
                    h_tc = sm_pool.tile([36, npc], BF16, tag="htc", name=f"htc_{li}")
                    nc.vector.tensor_tensor(
                        out=h_tc[:cin, :], in0=h_t[:cin, :], in1=cntrep_s[:cin, :],
                        op=ALU.mult)
                    lhst = h_tc[:cin, :]
                nc.tensor.matmul(agg[:, cc:cc + cout], lhsT=lhst, rhs=root_s[li],
                                 start=True, stop=False)
                nc.tensor.matmul(agg[:, cc:cc + cout], lhsT=cntrow_s, rhs=bias_s[li],
                                 start=False, stop=True)
                red = sm_pool.tile([npc, cout], F32, tag="red", name=f"red_{li}")
                nc.vector.tensor_reduce(
                    out=red, in_=_ap(agg, [(cin, cout), (1, cin)]),
                    axis=AXL.X, op=ALU.add)
                tot = sm_pool.tile([npc, cout], F32, tag="tot", name=f"tot_{li}")
                nc.vector.tensor_tensor(out=tot, in0=red, in1=agg[:, cc:cc + cout],
                                        op=ALU.add)
                h_s = sm_pool.tile([npc, cout], F32, tag="hs", name=f"hsf_{li}")
                nc.vector.tensor_scalar(h_s, tot, recip_s[:, 0:1], 0.0, ALU.mult, ALU.max)
                h_bf = sm_pool.tile([npc, cout], BF16, tag="hbf", name=f"hbf_{li}")
                nc.vector.tensor_scalar(h_bf, h_s, 1.0, None, ALU.mult)
                nc.sync.dma_start(out=h_loc[li][:, :], in_=h_bf)
                nc.gpsimd.collective_compute(
                    "AllGather", ALU.bypass,
                    replica_groups=[list(range(n_cores))],
                    ins=[h_loc[li].opt()], outs=[h_full[li].opt()])
                return h_bf

            # ================= layer 0 (cin=1: msg = relu(z)) =================
            cin, cout = DIMS[0]
            cc = cin * cout
            n_tri = n_chunks // 3
            for t in range(n_tri):
                zt = z_pool.tile([P, 864], F32, tag="z", name=f"z1_{t}")
                z3 = zt[:, 0:3 * cc]
                nc.tensor.matmul(z3, lhsT=ea_s[0:69, t * P:(t + 1) * P],
                                 rhs=lwbd_s[0], start=True, stop=True)
                p3 = p_pool.tile([P, 3 * cc], BF16, tag="p1", name=f"p1_{t}")
                if t % 2 == 0:
                    nc.vector.tensor_scalar(p3, z3, 0.0, None, ALU.max)
                else:
                    nc.scalar.activation(out=p3, in_=z3, func=RELU)
                for b in range(3):
                    c = 3 * t + b
                    nc.tensor.matmul(
                        agg[:, 0:cc], lhsT=mask_ap(c), rhs=p3[:, b * cc:(b + 1) * cc],
                        start=(c == 0), stop=(c == n_chunks - 1))
            h_prev_bf = epilogue(0, cin, cout, cc)

            # ================= layer 1 (the big one, cc=864) =================
            cin, cout = DIMS[1]
            cc = cin * cout
            # gathers (wait on AllGather via h_full dep)
            hs2 = []
            for sl in range(n_slabs):
                c0 = sl * SLAB
                c1 = min(n_chunks, c0 + SLAB)
                t = hs_pool.tile([P, SLAB, cin], BF16, tag="hs2", name=f"hs2_{sl}")
                nc.gpsimd.indirect_dma_start(
                    out=t[:, :c1 - c0, :], out_offset=None, in_=h_full[0][:, :],
                    in_offset=IndirectOffsetOnAxis(ap=offs_s[:, c0:c1], axis=0))
                hs2.append(t)

            def emit_z2(c):
                b, tg = c % 3, c // 3
                z = z_pool.tile([P, cc], F32, tag="z", name=f"z2_{c}")
                for off, n in SPLITS2:
                    nc.tensor.matmul(
                        z[:, off:off + n],
                        lhsT=ea_s[32 * b:32 * b + 5, tg * P:(tg + 1) * P],
                        rhs=lwr_s[1][32 * b:32 * b + 5, off:off + n],
                        start=True, stop=True)
                return z

            def emit_relu2(c, z, full):
                pre = pre2_pool.tile([P, cc], BF16, tag="pre2", name=f"pre2_{c}")
                o0 = 0 if full else STT2 * cin
                nc.scalar.activation(out=pre[:, o0:cc], in_=z[:, o0:cc], func=RELU)
                return pre

            zq, preq = {}, {}
            for c in range(min(NPRO2, n_chunks)):
                zq[c] = emit_z2(c)
                preq[c] = emit_relu2(c, zq[c], True)
            for c in range(n_chunks):
                full = c < NPRO2 + NCATCH2
                z, pre = zq.pop(c), preq.pop(c)
                hsv = hs2[c // SLAB][:, c % SLAB, :]
                h1 = _ap(hsv, [(0, cout), (1, cin)])
                p_t = p_pool.tile([P, cc], BF16, tag="p2", name=f"p2_{c}")
                if full:
                    nc.vector.tensor_tensor(
                        out=_ap(p_t, [(cin, cout), (1, cin)]),
                        in0=_ap(pre, [(cin, cout), (1, cin)]), in1=h1, op=ALU.mult)
                else:
                    nc.vector.scalar_tensor_tensor(
                        out=_ap(p_t, [(cin, STT2), (1, cin)]),
                        in0=_ap(z, [(cin, STT2), (1, cin)]), scalar=0.0,
                        in1=_ap(hsv, [(0, STT2), (1, cin)]),
                        op0=ALU.max, op1=ALU.mult)
                    o0 = STT2 * cin
                    nc.vector.tensor_tensor(
                        out=_apo(p_t, o0, [(cin, cout - STT2), (1, cin)]),
                        in0=_apo(pre, o0, [(cin, cout - STT2), (1, cin)]),
                        in1=_ap(hsv, [(0, cout - STT2), (1, cin)]), op=ALU.mult)
                for off, n in SPLITS2:
                    nc.tensor.matmul(
                        agg[:, off:off + n], lhsT=mask_ap(c), rhs=p_t[:, off:off + n],
                        start=(c == 0), stop=(c == n_chunks - 1))
                if c + NPRO2 < n_chunks:
                    c2 = c + NPRO2
                    zq[c2] = emit_z2(c2)
                    preq[c2] = emit_relu2(c2, zq[c2], c2 < NPRO2 + NCATCH2)
            h_prev_bf = epilogue(1, cin, cout, cc)

            # ================= layer 2 (cc=192, triples) =================
            cin, cout = DIMS[2]
            cc = cin * cout
            hs3 = []
            for sl in range(n_slabs):
                c0 = sl * SLAB
                c1 = min(n_chunks, c0 + SLAB)
                t = hs_pool.tile([P, SLAB, cin], BF16, tag="hs3", name=f"hs3_{sl}")
                nc.gpsimd.indirect_dma_start(
                    out=t[:, :c1 - c0, :], out_offset=None, in_=h_full[1][:, :],
                    in_offset=IndirectOffsetOnAxis(ap=offs_s[:, c0:c1], axis=0))
                hs3.append(t)

            def emit_z3(t):
                zt = z_pool.tile([P, 864], F32, tag="z", name=f"z3_{t}")
                z = zt[:, 0:3 * cc]
                for off, n in ((0, 512), (512, 3 * cc - 512)):
                    nc.tensor.matmul(z[:, off:off + n],
                                     lhsT=ea_s[0:69, t * P:(t + 1) * P],
                                     rhs=lwbd_s[2][:, off:off + n],
                                     start=True, stop=True)
                return z

            def emit_relu3(t, z, full):
                pre = pre3_pool.tile([P, 3 * cc], BF16, tag="pre3", name=f"pre3_{t}")
                if full:
                    nc.scalar.activation(out=pre, in_=z, func=RELU)
                else:
                    o0 = STT3 * cin
                    nc.scalar.activation(
                        out=_apo(pre, o0, [(cc, 3), (1, cc - o0)]),
                        in_=_apo(z, o0, [(cc, 3), (1, cc - o0)]),
                        func=RELU)
                return pre

            zq3, preq3 = {}, {}
            for t in range(min(NPRO3, n_tri)):
                zq3[t] = emit_z3(t)
                preq3[t] = emit_relu3(t, zq3[t], True)
            for t in range(n_tri):
                full = t < NPRO3 + NCATCH3
                z, pre = zq3.pop(t), preq3.pop(t)
                c0 = 3 * t
                sl, si = c0 // SLAB, c0 % SLAB
                # hs for chunks c0..c0+2 may cross a slab boundary
                p_t = p_pool.tile([P, 3 * cc], BF16, tag="p3", name=f"p3_{t}")
                for b in range(3):
                    c = c0 + b
                    hsv = hs3[c // SLAB][:, c % SLAB, :]
                    if full:
                        nc.vector.tensor_tensor(
                            out=_apo(p_t, b * cc, [(cin, cout), (1, cin)]),
                            in0=_apo(pre, b * cc, [(cin, cout), (1, cin)]),
                            in1=_ap(hsv, [(0, cout), (1, cin)]), op=ALU.mult)
                    else:
                        o0 = STT3 * cin
                        nc.vector.scalar_tensor_tensor(
                            out=_apo(p_t, b * cc, [(cin, STT3), (1, cin)]),
                            in0=_apo(z, b * cc, [(cin, STT3), (1, cin)]),
                            scalar=0.0, in1=_ap(hsv, [(0, STT3), (1, cin)]),
                            op0=ALU.max, op1=ALU.mult)
                        nc.vector.tensor_tensor(
                            out=_apo(p_t, b * cc + o0, [(cin, cout - STT3), (1, cin)]),
                            in0=_apo(pre, b * cc + o0, [(cin, cout - STT3), (1, cin)]),
                            in1=_ap(hsv, [(0, cout - STT3), (1, cin)]), op=ALU.mult)
                for b in range(3):
                    c = c0 + b
                    nc.tensor.matmul(
                        agg[:, 0:cc], lhsT=mask_ap(c), rhs=p_t[:, b * cc:(b + 1) * cc],
                        start=(c == 0), stop=(c == n_chunks - 1))
                if t + NPRO3 < n_tri:
                    t2 = t + NPRO3
                    zq3[t2] = emit_z3(t2)
                    preq3[t2] = emit_relu3(t2, zq3[t2], t2 < NPRO3 + NCATCH3)
            h_prev_bf = epilogue(2, cin, cout, cc)

            # ================= CBT =================
            dlast = DIMS[-1][1]
            hrep = sm_pool.tile([npc, nn, dlast], BF16, tag="hrep")
            nc.sync.dma_start(
                out=hrep.rearrange("p j d -> p (j d)"),
                in_=_ap(h_full[-1][:, :], [(1, nn * dlast)], pdim=(0, npc)))
            diff = sm_pool.tile([npc, nn, dlast], BF16, tag="diff")
            nc.vector.tensor_tensor(
                out=diff[:], in0=hrep[:],
                in1=_ap(h_prev_bf[:], [(0, nn), (1, dlast)]), op=ALU.subtract)
            cbt = sm_pool.tile([npc, nn], F32, tag="cbt")
            nc.vector.tensor_reduce(
                out=cbt, in_=diff[:], axis=AXL.X, op=ALU.add,
                apply_absolute_value=True)
            nc.sync.dma_start(out=out_d[:, :], in_=cbt[:])
    return nc


def _make_in_maps(inputs, cores):
    x = np.asarray(inputs["x"], np.float32)
    x0 = float(x[0, 0])
    in_maps = []
    for c in range(len(cores)):
        m = dict(
            ea69=cores[c]["ea69"], offs_t=cores[c]["offs_t"],
            masks_t=cores[c]["masks_t"], recip=cores[c]["recip"],
            cntrow=cores[c]["cntrow"], cntrep=cores[c]["cntrep"],
            xcnt=cores[c]["xcnt"],
        )
        for li, (cin, cout) in enumerate(DIMS):
            cc = cin * cout
            lw = np.asarray(inputs[f"lin_w{li + 1}"], np.float32)
            lb = np.asarray(inputs[f"lin_b{li + 1}"], np.float32)
            lw5 = np.vstack([lw, lb[None, :]]).astype(np.float32)
            if li == 0:
                lw5 = lw5 * x0
            lw5 = _perm_oi(lw5, cin, cout).astype(NPBF)
            lwr = np.zeros((69, cc), dtype=NPBF)
            lwbd = np.zeros((69, 3 * cc), dtype=NPBF)
            for b in range(3):
                lwr[32 * b:32 * b + 5, :] = lw5
                lwbd[32 * b:32 * b + 5, b * cc:(b + 1) * cc] = lw5
            m[f"lwr_{li}"] = lwr
            m[f"lwbd_{li}"] = lwbd
            m[f"root_{li}"] = np.asarray(inputs[f"root{li + 1}"], np.float32).astype(NPBF)
            m[f"bias_{li}"] = np.asarray(
                inputs[f"bias{li + 1}"], np.float32).reshape(1, -1).astype(NPBF)
        in_maps.append(m)
    return in_maps


def _run(inputs, n_cores, sim=False):
    x = np.asarray(inputs["x"], np.float32)
    nn = x.shape[0]
    x0 = float(x[0, 0])
    assert np.all(x == x0) and x0 >= 0.0, "general-x path not implemented"
    cores, n_chunks = _host_prep(x, inputs["edge_attr"], inputs["edge_index"], n_cores)
    nc = _build_program(nn, n_cores, n_chunks)
    nc.finalize()
    in_maps = _make_in_maps(inputs, cores)

    global _LAST
    _LAST = (nc, in_maps)
    if sim:
        from concourse.bass_interp import MultiCoreSim

        ms = MultiCoreSim(nc, n_cores)
        for c in range(n_cores):
            for k, v in in_maps[c].items():
                ms.cores[c].tensor(k)[:] = v
        ms.simulate()
        rows = [np.asarray(ms.cores[c].tensor("out_cbt")) for c in range(n_cores)]
    else:
        res = run_bass_kernel_spmd(nc, in_maps, list(range(n_cores)))
        rows = [res.results[c]["out_cbt"] for c in range(n_cores)]
    return np.concatenate(rows, 0).astype(np.float32)


_LAST = None


def kernel(**inputs) -> np.ndarray:
    return _run(inputs, n_cores=8, sim=False)
